# revision 13
# baseline (speedup 1.0000x reference)
"""Bahdanau-attention kernel for TRN2, SPMD over 8 NeuronCores.

Problem (hardcoded): B=32, N=4096, ENC=1024, DEC=512, ATT=512, f32.
  att1 = enc @ W_enc + b_enc            [B, N, ATT]
  att2 = dec @ W_dec + b_dec            [B, 1, ATT]
  att  = relu(att1 + att2)
  e    = att @ w_full + b_full          [B, N]
  alpha = softmax(e, axis=1)
  awe  = einsum('bne,bn->be', enc, alpha)
Returns (awe, alpha).

Sharding: data-parallel over batch, 4 items per core; weights replicated.
b_full is dropped: softmax is invariant to adding a constant.

Per-core dataflow (per batch item):
  - stream enc[b] in 8 tiles of [128p, 4, 1024] f32 (2 MiB DMAs),
    cast to a SBUF-resident bf16 copy (ScalarE),
  - PE-transpose 128x128 bf16 blocks -> eoT [enc, seq] tiles,
  - att1^T [att, seq] matmuls (W_enc bf16 stationary), relu+bias fused on
    ScalarE (bias = b_enc + b_dec + dec@W_dec precomputed per batch),
  - e row via PE contraction with w_full, evac to SBUF,
  - softmax: DVE max (negated), ScalarE exp with accumulated sum,
  - alpha^T via PE transposes; awe = sum_n p_n * enc[n] on PE using the
    resident bf16 copy (PSUM-accumulated over 32 seq chunks), scaled by 1/sum.
"""

import sys

sys.path.insert(0, "/opt/trn_rl_repo")

from contextlib import ExitStack

import numpy as np

import concourse.bass as bass
import concourse.tile as tile
from concourse import bacc, mybir
from concourse.bass_utils import run_bass_kernel_spmd
from concourse.masks import make_identity

F32 = mybir.dt.float32
BF16 = mybir.dt.bfloat16
AF = mybir.ActivationFunctionType

B, N, ENC, DEC, ATT = 32, 4096, 1024, 512, 512
NCORES = 8
BL = B // NCORES  # batch items per core
NSEQT = N // 512  # seq tiles of 512 per batch item (8)
NCH = ENC // 128  # enc chunks (8)
NATT = ATT // 128  # att tiles (4)
NDEC = DEC // 128  # dec chunks (4)


def _kernel_body(ctx: ExitStack, tc: tile.TileContext, io: dict):
    import os

    STAGE = int(os.environ.get("KSTAGE", "9"))
    nc = tc.nc

    singles = ctx.enter_context(tc.tile_pool(name="singles", bufs=1))
    stage = ctx.enter_context(tc.tile_pool(name="stage", bufs=2))
    eo_res = ctx.enter_context(tc.tile_pool(name="eo_res", bufs=NSEQT + 1))
    eot_pool = ctx.enter_context(tc.tile_pool(name="eot", bufs=16))
    attr_pool = ctx.enter_context(tc.tile_pool(name="attr", bufs=8))
    ebuf_pool = ctx.enter_context(tc.tile_pool(name="ebuf", bufs=2))
    small = ctx.enter_context(tc.tile_pool(name="small", bufs=4))
    out_pool = ctx.enter_context(tc.tile_pool(name="outs", bufs=2))
    dram_pool = ctx.enter_context(tc.tile_pool(name="dram", bufs=2, space="DRAM"))

    psum_t = ctx.enter_context(tc.tile_pool(name="psum_t", bufs=2, space="PSUM"))
    psum_att = ctx.enter_context(tc.tile_pool(name="psum_att", bufs=2, space="PSUM"))
    psum_e = ctx.enter_context(tc.tile_pool(name="psum_e", bufs=1, space="PSUM"))
    psum_awe = ctx.enter_context(tc.tile_pool(name="psum_awe", bufs=1, space="PSUM"))
    psum_misc = ctx.enter_context(tc.tile_pool(name="psum_misc", bufs=1, space="PSUM"))

    # ---- constants / weights ----
    ident = singles.tile([128, 128], F32)
    make_identity(nc, ident)

    # W_enc chunks: [128 enc, 512 att] f32 -> bf16
    w_enc_bf = []
    for c in range(NCH):
        wf = stage.tile([128, ATT], F32, tag="wstage")
        nc.sync.dma_start(out=wf, in_=io["W_enc"][c * 128 : (c + 1) * 128, :])
        wb = singles.tile([128, ATT], BF16, tag=f"wencbf{c}")
        nc.scalar.copy(out=wb, in_=wf)
        w_enc_bf.append(wb)

    # W_dec chunks: [128 dec, 512 att] f32
    w_dec = []
    for c in range(NDEC):
        wd = singles.tile([128, ATT], F32, tag=f"wdec{c}")
        nc.sync.dma_start(out=wd, in_=io["W_dec"][c * 128 : (c + 1) * 128, :])
        w_dec.append(wd)

    # biases, transposed layout [128, NATT]: elem (p, t) = v[t*128 + p]
    b_enc_t = singles.tile([128, NATT], F32)
    nc.sync.dma_start(out=b_enc_t, in_=io["b_enc"].rearrange("(t p) -> p t", p=128))
    b_dec_t = singles.tile([128, NATT], F32)
    nc.sync.dma_start(out=b_dec_t, in_=io["b_dec"].rearrange("(t p) -> p t", p=128))
    bsum = singles.tile([128, NATT], F32)
    nc.vector.tensor_add(bsum, b_enc_t, b_dec_t)

    # w_full transposed [128, NATT] bf16
    w_full_t = singles.tile([128, NATT], F32)
    nc.sync.dma_start(out=w_full_t, in_=io["w_full"].rearrange("(t p) -> p t", p=128))
    w_full_bf = singles.tile([128, NATT], BF16)
    nc.vector.tensor_copy(out=w_full_bf, in_=w_full_t)

    # decoder hidden, transposed: [128 dec-part, NDEC chunk, BL batch]
    dec_t = singles.tile([128, NDEC, BL], F32)
    dec_r = io["dec_in"].rearrange("b (c p) -> p c b", p=128)
    for c in range(NDEC):
        nc.sync.dma_start(out=dec_t[:, c, :], in_=dec_r[:, c, :])

    # att2^T per att tile: [128 att, BL], = dec @ W_dec + b_enc + b_dec
    att2 = []
    for t in range(NATT):
        ps = psum_misc.tile([128, BL], F32, tag="miscps")
        for c in range(NDEC):
            nc.tensor.matmul(
                ps,
                lhsT=w_dec[c][:, t * 128 : (t + 1) * 128],
                rhs=dec_t[:, c, :],
                start=(c == 0),
                stop=(c == NDEC - 1),
            )
        a2 = singles.tile([128, BL], F32, tag=f"att2_{t}")
        nc.vector.tensor_scalar_add(a2, ps, bsum[:, t : t + 1])
        att2.append(a2)

    # ---- main loop over local batch items ----
    for b in range(BL if STAGE >= 1 else 0):
        eo_bf_tiles = []
        e_buf = ebuf_pool.tile([1, N], F32)

        for st in range(NSEQT):
            n0 = st * 512
            # load [128, 4, 1024] f32: (p, ss, e) = enc[b, n0 + ss*128 + p, e]
            stg = stage.tile([128, 4, ENC], F32, tag="eostage")
            nc.sync.dma_start(out=stg, in_=io["enc_in"][b, n0 : n0 + 512, :].rearrange("(ss p) e -> p ss e", p=128))
            eo_bf = eo_res.tile([128, 4, ENC], BF16, tag="eobf")
            nc.scalar.copy(out=eo_bf, in_=stg)
            eo_bf_tiles.append(eo_bf)
            if STAGE < 2:
                continue

            # transpose: eoT[c] = [128 enc, 512 seq], f32 PE transpose + bf16 evac
            eot_tiles = []
            for c in range(NCH):
                pst = psum_t.tile([128, 512], F32, tag="pst")
                for ss in range(4):
                    nc.tensor.transpose(
                        pst[:, ss * 128 : (ss + 1) * 128],
                        stg[:, ss, c * 128 : (c + 1) * 128],
                        ident,
                    )
                eot = eot_pool.tile([128, 512], BF16, tag="eot")
                nc.vector.tensor_copy(out=eot, in_=pst)
                eot_tiles.append(eot)

            if STAGE < 3:
                continue
            # att1^T tiles + fused bias/relu -> attR (bf16)
            attr_tiles = []
            for t in range(NATT):
                psa = psum_att.tile([128, 512], F32, tag="psa")
                for c in range(NCH):
                    nc.tensor.matmul(
                        psa,
                        lhsT=w_enc_bf[c][:, t * 128 : (t + 1) * 128],
                        rhs=eot_tiles[c],
                        start=(c == 0),
                        stop=(c == NCH - 1),
                    )
                attr = attr_pool.tile([128, 512], BF16, tag="attr")
                nc.scalar.activation(
                    out=attr, in_=psa, func=AF.Relu, bias=att2[t][:, b : b + 1], scale=1.0
                )
                attr_tiles.append(attr)

            if STAGE < 4:
                continue
            # e row segment [1, 512]
            pse = psum_e.tile([1, 512], F32, tag="pse")
            for t in range(NATT):
                nc.tensor.matmul(
                    pse,
                    lhsT=w_full_bf[:, t : t + 1],
                    rhs=attr_tiles[t],
                    start=(t == 0),
                    stop=(t == NATT - 1),
                )
            nc.vector.tensor_copy(out=e_buf[:, n0 : n0 + 512], in_=pse)

        if STAGE < 4:
            nc.vector.memset(e_buf, 1.0)
        # ---- softmax over [1, N] ----
        neg_m = small.tile([1, 1], F32, tag="negm")
        nc.vector.tensor_reduce(
            out=neg_m, in_=e_buf, axis=mybir.AxisListType.X, op=mybir.AluOpType.max,
            negate=True,
        )
        ssum = small.tile([1, 1], F32, tag="ssum")
        # p = exp(e - m) in place; ssum = sum(p)
        nc.scalar.activation(
            out=e_buf, in_=e_buf, func=AF.Exp, bias=neg_m[0:1, 0:1], scale=1.0,
            accum_out=ssum,
        )
        rs = small.tile([1, 1], F32, tag="rs")
        nc.vector.reciprocal(out=rs, in_=ssum)

        # normalize alpha in place and store it; also bounce through DRAM to
        # obtain alpha^T [128 seq-part, 32 chunks] (SBUF APs cannot move data
        # across partitions, DMA via DRAM can).
        nc.vector.tensor_scalar_mul(e_buf, e_buf, rs[0:1, 0:1])
        nc.sync.dma_start(out=io["alpha_out"][b : b + 1, :], in_=e_buf)
        p_t_bf = small.tile([128, 32], BF16, tag="ptbf")
        if STAGE >= 5:
            a_dram = dram_pool.tile([1, N], F32, tag="adram")
            nc.sync.dma_start(out=a_dram, in_=e_buf)
            p_t = small.tile([128, 32], F32, tag="pt")
            nc.sync.dma_start(
                out=p_t, in_=a_dram.rearrange("o (k p) -> p (o k)", p=128)
            )
            nc.vector.tensor_copy(out=p_t_bf, in_=p_t)
        else:
            nc.vector.memset(p_t_bf, 0.001)

        # awe = (sum_n p_n * enc[b, n, :]) * rs
        psw = psum_awe.tile([1, ENC], F32, tag="psw")
        if STAGE < 6:
            nc.vector.memset(psw, 0.0)
        for st in range(NSEQT if STAGE >= 6 else 0):
            for ss in range(4):
                k = st * 4 + ss
                for h in range(2):
                    nc.tensor.matmul(
                        psw[0:1, h * 512 : (h + 1) * 512],
                        lhsT=p_t_bf[:, k : k + 1],
                        rhs=eo_bf_tiles[st][:, ss, h * 512 : (h + 1) * 512],
                        start=(k == 0),
                        stop=(k == N // 128 - 1),
                    )
        awe_sb = out_pool.tile([1, ENC], F32, tag="awe")
        nc.vector.tensor_copy(out=awe_sb, in_=psw)
        nc.sync.dma_start(out=io["awe_out"][b : b + 1, :], in_=awe_sb)
    if STAGE < 1:
        _stage0_outputs(tc, io, out_pool)


def _stage0_outputs(tc, io, pool):
    nc = tc.nc
    for b in range(BL):
        t = pool.tile([1, ENC], F32, tag="awe")
        nc.vector.memset(t, 0.5)
        nc.sync.dma_start(out=io["awe_out"][b : b + 1, :], in_=t)
        t2 = pool.tile([1, N], F32, tag="alph")
        nc.vector.memset(t2, 0.25)
        nc.sync.dma_start(out=io["alpha_out"][b : b + 1, :], in_=t2)


_CACHE: dict = {}


def _build():
    if "nc" in _CACHE:
        return _CACHE["nc"]
    nc = bacc.Bacc(
        "TRN2", target_bir_lowering=False, debug=False, num_devices=NCORES
    )
    io = {
        "enc_in": nc.dram_tensor("enc_in", [BL, N, ENC], F32, kind="ExternalInput").ap(),
        "dec_in": nc.dram_tensor("dec_in", [BL, DEC], F32, kind="ExternalInput").ap(),
        "W_enc": nc.dram_tensor("W_enc", [ENC, ATT], F32, kind="ExternalInput").ap(),
        "b_enc": nc.dram_tensor("b_enc", [ATT], F32, kind="ExternalInput").ap(),
        "W_dec": nc.dram_tensor("W_dec", [DEC, ATT], F32, kind="ExternalInput").ap(),
        "b_dec": nc.dram_tensor("b_dec", [ATT], F32, kind="ExternalInput").ap(),
        "w_full": nc.dram_tensor("w_full", [ATT], F32, kind="ExternalInput").ap(),
        "awe_out": nc.dram_tensor("awe_out", [BL, ENC], F32, kind="ExternalOutput").ap(),
        "alpha_out": nc.dram_tensor("alpha_out", [BL, N], F32, kind="ExternalOutput").ap(),
    }
    with tile.TileContext(nc) as tc:
        with ExitStack() as ctx:
            _kernel_body(ctx, tc, io)
    nc.compile()
    _CACHE["nc"] = nc
    return nc


def kernel(**inputs) -> tuple[np.ndarray, np.ndarray]:
    nc = _build()
    enc = np.asarray(inputs["encoder_out"], dtype=np.float32)
    dec = np.asarray(inputs["decoder_hidden"], dtype=np.float32)
    shared = {
        "W_enc": np.asarray(inputs["W_enc"], np.float32),
        "b_enc": np.asarray(inputs["b_enc"], np.float32),
        "W_dec": np.asarray(inputs["W_dec"], np.float32),
        "b_dec": np.asarray(inputs["b_dec"], np.float32),
        "w_full": np.asarray(inputs["w_full"], np.float32),
    }
    in_maps = []
    for i in range(NCORES):
        sl = slice(i * BL, (i + 1) * BL)
        in_maps.append({"enc_in": enc[sl], "dec_in": dec[sl], **shared})
    res = run_bass_kernel_spmd(nc, in_maps, core_ids=list(range(NCORES)))
    awe = np.concatenate([res.results[i]["awe_out"] for i in range(NCORES)], axis=0)
    alpha = np.concatenate(
        [res.results[i]["alpha_out"] for i in range(NCORES)], axis=0
    )
    return awe, alpha
